# revision 21
# baseline (speedup 1.0000x reference)
"""3-layer GAT on 8 Trainium2 NeuronCores.

Strategy (dst-sharded, stripe-pipelined):
- Core k owns destination nodes [6250k, 6250(k+1)); edges partitioned by dst
  owner, grouped into 49 blocks of 128 dst nodes.
- Node table rows are [feat | el | pad] in bf16 and live in DRAM, laid out
  stripe-major: stripe j holds rows j*C..(j+1)*C of every core's slice, so a
  sub-AllGather per stripe writes one contiguous region and region-relative
  gather indices stay < 12504 (int16).
- Per layer: feat/el/er come from ONE matmul per node tile with extended
  weights [W | W@al | W@ar]; slices are written per stripe chunk and
  sub-AllGathered as soon as their chunk of nodes is done, overlapping the
  collective with edge compute of the previous blocks/layer.
- Edge phase runs in groups of 4 blocks; per (group, stripe): one batched
  dma_gather of all source rows, host-precomputed one-hot tiles (edge-major
  ohE and transposed ohT, bf16) stream from DRAM, er[dst] expands per edge
  via ohT @ er matmuls, scores exponentiate on the Act engine into an
  expanded exR, ones are memset into the gathered el columns, and a single
  full-tile bf16 multiply forms [ex*feat | ex] that ohE^T scatter-matmuls
  into per-block PSUM accumulators ([unnorm | denom]). Softmax
  max-subtraction is dropped (scores are O(1); softmax is shift-invariant).
"""
import numpy as np

N = 50000
E = 500000
NC = 8
NLOC = N // NC          # 6250
P = 128
NBT = 49                # node tiles / blocks per core (48*128 + 106)
LAST_ROWS = NLOC - 48 * P   # 106
J = 4                   # stripes (sub-AllGather chunks)
C = 1563                # node rows per core per stripe (ceil(6250/4))
RC = NC * C             # table rows per stripe region (12504)
NTAB = J * RC           # 50016
SLICE_R = J * C         # 6252 (slice rows, padded)
G = 1                   # edge-phase block group size
IN = 128
D = 256
H12 = 4
F = 64
CLS = 64
TW12 = 384              # table row bf16 words (256 feat + 4 el + pad) -> 768B
TW3 = 128               # (64 feat + 1 el + pad) -> 256B
SLOPE = 0.2

GROUPS = [list(range(g, min(g + G, NBT))) for g in range(0, NBT, G)]
AGTRIG = {12: 0, 24: 1, 36: 2, 48: 3}   # feat block -> stripe chunk ready


def _wrap_idx16(ix):
    """[n*128] int16 -> dma_gather wrapped layout [128, n*8]."""
    n = len(ix) // P
    return np.tile(ix.reshape(n * 8, 16).T, (8, 1)).astype(np.int16)


def make_schedule(src, dst):
    """Uniform (across cores) tile schedule + per-core index/position data.

    Tile order: for each block-group g, for each stripe j, for each block b
    in g, that (b, j)'s tiles — so every (g, j) is one contiguous run."""
    src = np.asarray(src).astype(np.int64)
    dst = np.asarray(dst).astype(np.int64)
    owner = dst // NLOC
    per_core = []
    cnt = np.zeros((NC, NBT, J), np.int64)
    for k in range(NC):
        m = owner == k
        s = src[m]
        dl = dst[m] - k * NLOC
        blk = dl // P
        pos = dl % P
        stripe = (s % NLOC) // C
        order = np.lexsort((stripe, blk))
        per_core.append((s[order], blk[order], pos[order], stripe[order]))
        for b in range(NBT):
            mb = blk[order] == b
            jb = stripe[order][mb]
            for j in range(J):
                cnt[k, b, j] = int((jb == j).sum())

    Tbj = np.ceil(cnt / P).astype(int).max(axis=0)       # [NBT, J]

    tile_block = []
    tile_stripe = []
    for grp in GROUPS:
        for j in range(J):
            for b in grp:
                tile_block += [b] * Tbj[b, j]
                tile_stripe += [j] * Tbj[b, j]
    TT = len(tile_block)

    # tile id ranges per (b, j)
    tid = {}
    t0 = 0
    for grp in GROUPS:
        for j in range(J):
            for b in grp:
                tid[(b, j)] = (t0, Tbj[b, j])
                t0 += Tbj[b, j]

    idx16 = np.zeros((NC, P, TT * 8), np.int16)
    dstpos = np.full((NC, P, TT), 255.0, np.float32)
    for k in range(NC):
        s, blk, pos, stripe = per_core[k]
        for b in range(NBT):
            for j in range(J):
                t0, Tn = tid[(b, j)]
                if Tn == 0:
                    continue
                sel = (blk == b) & (stripe == j)
                ss = s[sel]
                rr = (ss // NLOC) * C + (ss % NLOC) - j * C   # region-relative
                pp = pos[sel]
                nfull = len(ss)
                buf_i = np.zeros(Tn * P, np.int16)
                buf_p = np.full(Tn * P, 255.0, np.float32)
                buf_i[:nfull] = rr.astype(np.int16)
                buf_p[:nfull] = pp.astype(np.float32)
                idx16[k, :, t0 * 8:(t0 + Tn) * 8] = _wrap_idx16(buf_i)
                for t in range(Tn):
                    dstpos[k, :, t0 + t] = buf_p[t * P:(t + 1) * P]
    return tile_block, tile_stripe, TT, idx16, dstpos


def build_nc(tile_block, tile_stripe, TT, n_layers=3):
    import concourse.bacc as bacc
    import concourse.bass as bass
    import concourse.mybir as mybir
    import concourse.tile as tile
    from concourse.library_config import mlp
    dt = mybir.dt

    # tile ranges per (b, j), per (g, j), and per g
    tid = {}
    gjr = {}
    t0 = 0
    for gi, grp in enumerate(GROUPS):
        for j in range(J):
            g0 = t0
            for b in grp:
                n = sum(1 for t in range(len(tile_block))
                        if tile_block[t] == b and tile_stripe[t] == j)
                tid[(b, j)] = (t0, n)
                t0 += n
            gjr[(gi, j)] = (g0, t0 - g0)
    grange = {}
    for gi, grp in enumerate(GROUPS):
        g0 = gjr[(gi, 0)][0]
        g1 = gjr[(gi, J - 1)][0] + gjr[(gi, J - 1)][1]
        grange[gi] = (g0, g1 - g0)
    # first/last tile id per block (for PSUM start/stop)
    bfirst = {}
    blast = {}
    for b in range(NBT):
        ids = []
        for j in range(J):
            s0, n = tid[(b, j)]
            ids += list(range(s0, s0 + n))
        bfirst[b] = min(ids)
        blast[b] = max(ids)

    nc = bacc.Bacc("TRN2", target_bir_lowering=False, debug=False,
                   num_devices=NC, num_swdge_queues=4)

    xT = nc.declare_dram_parameter("xT", [IN, NBT * P], dt.bfloat16, isOutput=False)
    w1 = nc.declare_dram_parameter("w1", [IN, D + 8], dt.bfloat16, isOutput=False)
    w2 = nc.declare_dram_parameter("w2", [D, D + 8], dt.bfloat16, isOutput=False)
    w3 = nc.declare_dram_parameter("w3", [D, CLS + 2 + CLS], dt.bfloat16, isOutput=False)
    idx_in = nc.declare_dram_parameter("idx16", [P, TT * 8], dt.int16, isOutput=False)
    ohE_in = nc.declare_dram_parameter("ohE", [P, TT * P], dt.bfloat16, isOutput=False)
    ohT_in = nc.declare_dram_parameter("ohT", [P, TT * P], dt.bfloat16, isOutput=False)
    ident_in = nc.declare_dram_parameter("ident", [P, P], dt.bfloat16, isOutput=False)
    outp = nc.declare_dram_parameter("out", [NLOC, CLS], dt.float32, isOutput=True)

    slices = [nc.dram_tensor(f"slice_{l}", [SLICE_R, TW12 if l < 2 else TW3],
                             dt.bfloat16) for l in range(3)]
    tables = [[nc.dram_tensor(f"table_{l}_{j}", [RC, TW12 if l < 2 else TW3],
                              dt.bfloat16, addr_space="Shared")
               for j in range(J)] for l in range(3)]

    groups_rg = [list(range(NC))]

    with tile.TileContext(nc) as tc:
        with (
            tc.tile_pool(name="pers", bufs=1) as pers,
            tc.tile_pool(name="kt", bufs=3) as ktp,
            tc.tile_pool(name="stage", bufs=3) as stp,
            tc.tile_pool(name="gblk", bufs=2) as gp,
            tc.tile_pool(name="ohblk", bufs=2) as ohp,
            tc.tile_pool(name="small", bufs=3) as smp,
            tc.tile_pool(name="exR", bufs=2) as xp,
            tc.tile_pool(name="vals", bufs=5) as vp,
            tc.tile_pool(name="otile", bufs=2) as op_,
            tc.tile_pool(name="ps_feat", bufs=2, space="PSUM") as psf,
            tc.tile_pool(name="ps_out", bufs=2, space="PSUM") as pso,
            tc.tile_pool(name="ps_tr", bufs=2, space="PSUM") as pstr,
            tc.tile_pool(name="ps_er", bufs=2, space="PSUM") as pser,
        ):
            nc.gpsimd.load_library(mlp)
            # persistent SBUF state
            xT_sb = pers.tile([P, NBT * P], dt.bfloat16)
            nc.sync.dma_start(xT_sb[:], xT[:])
            w1_sb = pers.tile([P, D + 8], dt.bfloat16)
            nc.sync.dma_start(w1_sb[:], w1[:])
            w2_sb = pers.tile([P, 2 * (D + 8)], dt.bfloat16)
            w3_sb = pers.tile([P, 2 * (CLS + 2 + CLS)], dt.bfloat16)
            for kt in range(2):
                nc.sync.dma_start(w2_sb[:, kt * (D + 8):(kt + 1) * (D + 8)],
                                  w2[kt * P:(kt + 1) * P, :])
                nc.sync.dma_start(w3_sb[:, kt * (CLS + 2 + CLS):(kt + 1) * (CLS + 2 + CLS)],
                                  w3[kt * P:(kt + 1) * P, :])
            idx_sb = pers.tile([P, TT * 8], dt.int16)
            nc.sync.dma_start(idx_sb[:], idx_in[:])
            ident_sb = pers.tile([P, P], dt.bfloat16)
            nc.sync.dma_start(ident_sb[:], ident_in[:])
            h_sb = pers.tile([P, NBT * D], dt.bfloat16)
            er_sb = pers.tile([P, NBT * H12], dt.bfloat16)
            er3_sb = pers.tile([P, NBT], dt.bfloat16)
            res_sb = pers.tile([P, NBT * CLS], dt.float32)

            def _elu_into(x_t, dst_sb, b, width):
                # elu(x) = max(x, exp(min(x,0)) - 1)
                t1 = op_.tile([P, width], dt.float32, tag="elu1")
                nc.vector.tensor_scalar_min(t1[:], x_t[:], 0.0)
                nc.scalar.activation(t1[:], t1[:], mybir.ActivationFunctionType.Exp)
                nc.vector.tensor_scalar_add(t1[:], t1[:], -1.0)
                nc.vector.tensor_tensor(out=dst_sb[:, b * width:(b + 1) * width],
                                        in0=x_t[:], in1=t1[:], op=mybir.AluOpType.max)

            def feat_block(layer, nt):
                if layer == 0:
                    wsb, wcols, nk = w1_sb, D + 8, 1
                elif layer == 1:
                    wsb, wcols, nk = w2_sb, D + 8, 2
                else:
                    wsb, wcols, nk = w3_sb, CLS + 2 + CLS, 2
                f_ps = psf.tile([P, wcols], dt.float32, tag="fp")
                for kt in range(nk):
                    if layer == 0:
                        lhsT = xT_sb[:, nt * P:(nt + 1) * P]
                    else:
                        tr_ps = pstr.tile([P, P], dt.bfloat16, tag="trp")
                        nc.tensor.transpose(
                            tr_ps[:], h_sb[:, nt * D + kt * P: nt * D + (kt + 1) * P],
                            ident_sb[:])
                        ktile = ktp.tile([P, P], dt.bfloat16, tag="kt")
                        nc.scalar.activation(ktile[:], tr_ps[:],
                                             mybir.ActivationFunctionType.Copy)
                        lhsT = ktile[:]
                    nc.tensor.matmul(f_ps[:], lhsT, wsb[:, kt * wcols:(kt + 1) * wcols],
                                     start=(kt == 0), stop=(kt == nk - 1))
                rows = P if nt < NBT - 1 else LAST_ROWS
                if layer < 2:
                    st = stp.tile([P, D + H12], dt.bfloat16, tag="st")
                    nc.vector.tensor_copy(st[:], f_ps[:, 0:D + H12])
                    nc.vector.tensor_copy(er_sb[:, nt * H12:(nt + 1) * H12],
                                          f_ps[:, D + H12:D + 2 * H12])
                    nc.sync.dma_start(slices[layer][nt * P:nt * P + rows, 0:D + H12],
                                      st[0:rows, :])
                else:
                    st = stp.tile([P, CLS + 1], dt.bfloat16, tag="st3")
                    nc.vector.tensor_copy(st[:], f_ps[:, 0:CLS + 1])
                    nc.vector.tensor_copy(er3_sb[:, nt:nt + 1],
                                          f_ps[:, CLS + 1:CLS + 2])
                    nc.vector.tensor_copy(res_sb[:, nt * CLS:(nt + 1) * CLS],
                                          f_ps[:, CLS + 2:CLS + 2 + CLS])
                    nc.sync.dma_start(slices[2][nt * P:nt * P + rows, 0:CLS + 1],
                                      st[0:rows, :])

            def emit_ag(layer, j):
                nc.gpsimd.collective_compute(
                    "AllGather", mybir.AluOpType.bypass, replica_groups=groups_rg,
                    ins=[slices[layer][j * C:(j + 1) * C, :]],
                    outs=[tables[layer][j][:, :]])

            qn = [0]

            def edge_phase(layer):
                if layer < 2:
                    TW, FO, NH = TW12, D, H12
                    er_l = er_sb
                else:
                    TW, FO, NH = TW3, CLS, 1
                    er_l = er3_sb
                W2c = FO + NH          # vals row width
                tab = tables[layer]
                for gi, grp in enumerate(GROUPS):
                    gt0, gT = grange[gi]
                    ohE_g = ohp.tile([P, gT * P], dt.bfloat16, tag="ohE")
                    ohT_g = ohp.tile([P, gT * P], dt.bfloat16, tag="ohT")
                    nc.sync.dma_start(ohE_g[:], ohE_in[:, gt0 * P:(gt0 + gT) * P])
                    nc.sync.dma_start(ohT_g[:], ohT_in[:, gt0 * P:(gt0 + gT) * P])
                    outps = {}
                    for b in grp:
                        out_acc = pso.tile([P, W2c], dt.float32, tag="outp",
                                           name=f"outacc_{layer}_{gi}_{b}")
                        outps[b] = out_acc
                    vals_j = {}
                    for j in range(J):
                        j0, jT = gjr[(gi, j)]
                        if jT == 0:
                            continue
                        Gblk = gp.tile([P, jT * TW], dt.bfloat16, tag="G")
                        nc.gpsimd.dma_gather(
                            Gblk[:].rearrange("p (c e) -> p c e", c=jT),
                            tab[j][:, :],
                            idx_sb[:, j0 * 8:(j0 + jT) * 8],
                            jT * P, jT * P, TW, queue_num=qn[0] % 4,
                        )
                        qn[0] += 1
                        er_ps = pser.tile([P, jT * NH], dt.float32, tag="erp")
                        for b in grp:
                            s0, n = tid[(b, j)]
                            for t in range(s0, s0 + n):
                                lo = t - gt0
                                nc.tensor.matmul(
                                    er_ps[:, (t - j0) * NH:(t - j0 + 1) * NH],
                                    ohT_g[:, lo * P:(lo + 1) * P],
                                    er_l[:, b * NH:(b + 1) * NH],
                                    start=True, stop=True)
                        e_sb = smp.tile([P, jT * NH], dt.float32, tag="e")
                        nc.vector.tensor_tensor(
                            out=e_sb[:],
                            in0=Gblk[:].rearrange("p (t c) -> p t c", t=jT)[:, :, FO:FO + NH],
                            in1=er_ps[:, :jT * NH],
                            op=mybir.AluOpType.add,
                        )
                        es_sb = smp.tile([P, jT * NH], dt.float32, tag="es")
                        nc.vector.tensor_scalar_mul(es_sb[:], e_sb[:], SLOPE)
                        nc.vector.tensor_tensor(out=e_sb[:], in0=e_sb[:], in1=es_sb[:],
                                                op=mybir.AluOpType.max)
                        ex_sb = smp.tile([P, jT * NH], dt.float32, tag="ex")
                        nc.scalar.activation(ex_sb[:], e_sb[:],
                                             mybir.ActivationFunctionType.Exp)
                        exR = xp.tile([P, jT * W2c], dt.bfloat16, tag="xr")
                        nc.scalar.activation(
                            exR[:].rearrange("p (t c) -> p t c", t=jT)[:, :, 0:FO]
                                .rearrange("p t (h f) -> p t h f", h=NH),
                            ex_sb[:].rearrange("p (t h o) -> p t h o", t=jT, o=1)
                                .to_broadcast([P, jT, NH, F]),
                            mybir.ActivationFunctionType.Copy,
                        )
                        nc.scalar.activation(
                            exR[:].rearrange("p (t c) -> p t c", t=jT)[:, :, FO:FO + NH],
                            ex_sb[:].rearrange("p (t h) -> p t h", t=jT),
                            mybir.ActivationFunctionType.Copy,
                        )
                        nc.vector.memset(
                            Gblk[:].rearrange("p (t c) -> p t c", t=jT)[:, :, FO:FO + NH],
                            1.0)
                        vals = vp.tile([P, jT * W2c], dt.bfloat16, tag="v")
                        nc.vector.tensor_tensor(
                            out=vals[:].rearrange("p (t c) -> p t c", t=jT),
                            in0=Gblk[:].rearrange("p (t c) -> p t c", t=jT)[:, :, 0:W2c],
                            in1=exR[:].rearrange("p (t c) -> p t c", t=jT),
                            op=mybir.AluOpType.mult,
                        )
                        vals_j[j] = vals
                    # contiguous PSUM accumulation group per block
                    for b in grp:
                        for j in range(J):
                            j0, jT = gjr[(gi, j)]
                            s0, n = tid[(b, j)]
                            for t in range(s0, s0 + n):
                                lo = t - gt0
                                nc.tensor.matmul(
                                    outps[b][:],
                                    ohE_g[:, lo * P:(lo + 1) * P],
                                    vals_j[j][:, (t - j0) * W2c:(t - j0 + 1) * W2c],
                                    start=(t == bfirst[b]), stop=(t == blast[b]))
                    for b in grp:
                        out_ps = outps[b]
                        den = smp.tile([P, NH], dt.float32, tag="den")
                        nc.vector.tensor_scalar_max(den[:], out_ps[:, FO:FO + NH], 1e-30)
                        rec = smp.tile([P, NH], dt.float32, tag="rec")
                        nc.vector.reciprocal(rec[:], den[:])
                        o_t = op_.tile([P, FO], dt.float32, tag="ot")
                        nc.vector.tensor_tensor(
                            out=o_t[:].rearrange("p (h f) -> p h f", h=NH),
                            in0=out_ps[:, 0:FO].rearrange("p (h f) -> p h f", h=NH),
                            in1=rec[:].to_broadcast([P, NH, F]),
                            op=mybir.AluOpType.mult,
                        )
                        # layer tails
                        if layer == 0:
                            _elu_into(o_t, h_sb, b, D)
                        elif layer == 1:
                            pre = op_.tile([P, D], dt.float32, tag="pre")
                            nc.vector.tensor_tensor(out=pre[:], in0=o_t[:],
                                                    in1=h_sb[:, b * D:(b + 1) * D],
                                                    op=mybir.AluOpType.add)
                            _elu_into(pre, h_sb, b, D)
                        else:
                            lg = op_.tile([P, CLS], dt.float32, tag="lg")
                            nc.vector.tensor_tensor(out=lg[:], in0=o_t[:],
                                                    in1=res_sb[:, b * CLS:(b + 1) * CLS],
                                                    op=mybir.AluOpType.add)
                            rows = P if b < NBT - 1 else LAST_ROWS
                            nc.sync.dma_start(outp[b * P:b * P + rows, :], lg[0:rows, :])
                        if layer < 2:
                            feat_block(layer + 1, b)

            for b in range(NBT):
                feat_block(0, b)
            for j in range(J):
                emit_ag(0, j)
            for layer in range(n_layers):
                edge_phase(layer)
                if layer < 2:
                    for j in range(J):
                        emit_ag(layer + 1, j)

    nc.compile()
    return nc


LAST_RESULTS = None


def prepare(inputs):
    import ml_dtypes
    bf16 = ml_dtypes.bfloat16

    x = np.asarray(inputs["x"], np.float32)
    src = np.asarray(inputs["src"]).astype(np.int64)
    dst = np.asarray(inputs["dst"]).astype(np.int64)
    W1 = np.asarray(inputs["W1"], np.float32)
    W2 = np.asarray(inputs["W2"], np.float32)
    W3 = np.asarray(inputs["W3"], np.float32)
    res_W3 = np.asarray(inputs["res_W3"], np.float32)
    al1 = np.asarray(inputs["al1"], np.float32)
    ar1 = np.asarray(inputs["ar1"], np.float32)
    al2 = np.asarray(inputs["al2"], np.float32)
    ar2 = np.asarray(inputs["ar2"], np.float32)
    al3 = np.asarray(inputs["al3"], np.float32)
    ar3 = np.asarray(inputs["ar3"], np.float32)

    def ext(W, al, ar, nh, res=None):
        Wr = W.reshape(W.shape[0], nh, -1)
        wel = np.einsum("khf,hf->kh", Wr, al)
        wer = np.einsum("khf,hf->kh", Wr, ar)
        parts = [W, wel, wer] + ([res] if res is not None else [])
        return np.ascontiguousarray(np.concatenate(parts, axis=1), dtype=bf16)

    w1e = ext(W1, al1, ar1, H12)                 # [128, 264]
    w2e = ext(W2, al2, ar2, H12)                 # [256, 264]
    w3e = ext(W3, al3, ar3, 1, res_W3)           # [256, 130]

    tile_block, tile_stripe, TT, idx16, dstpos = make_schedule(src, dst)
    nc = build_nc(tile_block, tile_stripe, TT)

    ident = np.eye(P, dtype=bf16)

    in_maps = []
    for k in range(NC):
        xk = x[k * NLOC:(k + 1) * NLOC].T                     # [128, 6250]
        xk = np.pad(xk, ((0, 0), (0, NBT * P - NLOC)))
        dp = dstpos[k]                                        # [128, TT]
        ohE = (dp[:, :, None] == np.arange(P, dtype=np.float32)[None, None, :])
        ohE = ohE.astype(bf16)                                # [e, t, d]
        ohT = np.ascontiguousarray(ohE.transpose(2, 1, 0))    # [d, t, e]
        in_maps.append({
            "xT": np.ascontiguousarray(xk).astype(bf16),
            "w1": w1e, "w2": w2e, "w3": w3e,
            "idx16": np.ascontiguousarray(idx16[k]),
            "ohE": np.ascontiguousarray(ohE.reshape(P, len(tile_block) * P)),
            "ohT": ohT.reshape(P, len(tile_block) * P),
            "ident": ident,
        })
    return nc, in_maps


def kernel(**inputs):
    from concourse.bass_utils import run_bass_kernel_spmd

    nc, in_maps = prepare(inputs)
    res = run_bass_kernel_spmd(nc, in_maps, core_ids=list(range(NC)))
    global LAST_RESULTS
    LAST_RESULTS = res
    out = np.concatenate([res.results[k]["out"] for k in range(NC)], axis=0)
    return out.astype(np.float32)


# revision 22
# speedup vs baseline: 1.0585x; 1.0585x over previous
"""3-layer GAT on 8 Trainium2 NeuronCores.

Strategy (dst-sharded, stripe-pipelined):
- Core k owns destination nodes [6250k, 6250(k+1)); edges partitioned by dst
  owner, grouped into 49 blocks of 128 dst nodes.
- Node table rows are [feat | el | pad] in bf16 and live in DRAM, laid out
  stripe-major: stripe j holds rows j*C..(j+1)*C of every core's slice, so a
  sub-AllGather per stripe writes one contiguous region and region-relative
  gather indices stay < 12504 (int16).
- Per layer: feat/el/er come from ONE matmul per node tile with extended
  weights [W | W@al | W@ar]; slices are written per stripe chunk and
  sub-AllGathered as soon as their chunk of nodes is done, overlapping the
  collective with edge compute of the previous blocks/layer.
- Edge phase runs in groups of 4 blocks; per (group, stripe): one batched
  dma_gather of all source rows, host-precomputed one-hot tiles (edge-major
  ohE and transposed ohT, bf16) stream from DRAM, er[dst] expands per edge
  via ohT @ er matmuls, scores exponentiate on the Act engine into an
  expanded exR, ones are memset into the gathered el columns, and a single
  full-tile bf16 multiply forms [ex*feat | ex] that ohE^T scatter-matmuls
  into per-block PSUM accumulators ([unnorm | denom]). Softmax
  max-subtraction is dropped (scores are O(1); softmax is shift-invariant).
"""
import numpy as np

N = 50000
E = 500000
NC = 8
NLOC = N // NC          # 6250
P = 128
NBT = 49                # node tiles / blocks per core (48*128 + 106)
LAST_ROWS = NLOC - 48 * P   # 106
J = 4                   # stripes (sub-AllGather chunks)
C = 1563                # node rows per core per stripe (ceil(6250/4))
RC = NC * C             # table rows per stripe region (12504)
NTAB = J * RC           # 50016
SLICE_R = J * C         # 6252 (slice rows, padded)
G = 2                   # edge-phase block group size
IN = 128
D = 256
H12 = 4
F = 64
CLS = 64
TW12 = 384              # table row bf16 words (256 feat + 4 el + pad) -> 768B
TW3 = 128               # (64 feat + 1 el + pad) -> 256B
SLOPE = 0.2

GROUPS = [list(range(g, min(g + G, NBT))) for g in range(0, NBT, G)]
AGTRIG = {12: 0, 24: 1, 36: 2, 48: 3}   # feat block -> stripe chunk ready


def _wrap_idx16(ix):
    """[n*128] int16 -> dma_gather wrapped layout [128, n*8]."""
    n = len(ix) // P
    return np.tile(ix.reshape(n * 8, 16).T, (8, 1)).astype(np.int16)


def make_schedule(src, dst):
    """Uniform (across cores) tile schedule + per-core index/position data.

    Tile order: for each block-group g, for each stripe j, for each block b
    in g, that (b, j)'s tiles — so every (g, j) is one contiguous run."""
    src = np.asarray(src).astype(np.int64)
    dst = np.asarray(dst).astype(np.int64)
    owner = dst // NLOC
    per_core = []
    cnt = np.zeros((NC, NBT, J), np.int64)
    for k in range(NC):
        m = owner == k
        s = src[m]
        dl = dst[m] - k * NLOC
        blk = dl // P
        pos = dl % P
        stripe = (s % NLOC) // C
        order = np.lexsort((stripe, blk))
        per_core.append((s[order], blk[order], pos[order], stripe[order]))
        for b in range(NBT):
            mb = blk[order] == b
            jb = stripe[order][mb]
            for j in range(J):
                cnt[k, b, j] = int((jb == j).sum())

    Tbj = np.ceil(cnt / P).astype(int).max(axis=0)       # [NBT, J]

    tile_block = []
    tile_stripe = []
    for grp in GROUPS:
        for j in range(J):
            for b in grp:
                tile_block += [b] * Tbj[b, j]
                tile_stripe += [j] * Tbj[b, j]
    TT = len(tile_block)

    # tile id ranges per (b, j)
    tid = {}
    t0 = 0
    for grp in GROUPS:
        for j in range(J):
            for b in grp:
                tid[(b, j)] = (t0, Tbj[b, j])
                t0 += Tbj[b, j]

    idx16 = np.zeros((NC, P, TT * 8), np.int16)
    dstpos = np.full((NC, P, TT), 255.0, np.float32)
    for k in range(NC):
        s, blk, pos, stripe = per_core[k]
        for b in range(NBT):
            for j in range(J):
                t0, Tn = tid[(b, j)]
                if Tn == 0:
                    continue
                sel = (blk == b) & (stripe == j)
                ss = s[sel]
                rr = (ss // NLOC) * C + (ss % NLOC) - j * C   # region-relative
                pp = pos[sel]
                nfull = len(ss)
                buf_i = np.zeros(Tn * P, np.int16)
                buf_p = np.full(Tn * P, 255.0, np.float32)
                buf_i[:nfull] = rr.astype(np.int16)
                buf_p[:nfull] = pp.astype(np.float32)
                idx16[k, :, t0 * 8:(t0 + Tn) * 8] = _wrap_idx16(buf_i)
                for t in range(Tn):
                    dstpos[k, :, t0 + t] = buf_p[t * P:(t + 1) * P]
    return tile_block, tile_stripe, TT, idx16, dstpos


def build_nc(tile_block, tile_stripe, TT, n_layers=3):
    import concourse.bacc as bacc
    import concourse.bass as bass
    import concourse.mybir as mybir
    import concourse.tile as tile
    from concourse.library_config import mlp
    dt = mybir.dt

    # tile ranges per (b, j), per (g, j), and per g
    tid = {}
    gjr = {}
    t0 = 0
    for gi, grp in enumerate(GROUPS):
        for j in range(J):
            g0 = t0
            for b in grp:
                n = sum(1 for t in range(len(tile_block))
                        if tile_block[t] == b and tile_stripe[t] == j)
                tid[(b, j)] = (t0, n)
                t0 += n
            gjr[(gi, j)] = (g0, t0 - g0)
    grange = {}
    for gi, grp in enumerate(GROUPS):
        g0 = gjr[(gi, 0)][0]
        g1 = gjr[(gi, J - 1)][0] + gjr[(gi, J - 1)][1]
        grange[gi] = (g0, g1 - g0)
    # first/last tile id per block (for PSUM start/stop)
    bfirst = {}
    blast = {}
    for b in range(NBT):
        ids = []
        for j in range(J):
            s0, n = tid[(b, j)]
            ids += list(range(s0, s0 + n))
        bfirst[b] = min(ids)
        blast[b] = max(ids)

    nc = bacc.Bacc("TRN2", target_bir_lowering=False, debug=False,
                   num_devices=NC, num_swdge_queues=4)

    xT = nc.declare_dram_parameter("xT", [IN, NBT * P], dt.bfloat16, isOutput=False)
    w1 = nc.declare_dram_parameter("w1", [IN, D + 8], dt.bfloat16, isOutput=False)
    w2 = nc.declare_dram_parameter("w2", [D, D + 8], dt.bfloat16, isOutput=False)
    w3 = nc.declare_dram_parameter("w3", [D, CLS + 2 + CLS], dt.bfloat16, isOutput=False)
    idx_in = nc.declare_dram_parameter("idx16", [P, TT * 8], dt.int16, isOutput=False)
    ohE_in = nc.declare_dram_parameter("ohE", [P, TT * P], dt.bfloat16, isOutput=False)
    ohT_in = nc.declare_dram_parameter("ohT", [P, TT * P], dt.bfloat16, isOutput=False)
    ident_in = nc.declare_dram_parameter("ident", [P, P], dt.bfloat16, isOutput=False)
    outp = nc.declare_dram_parameter("out", [NLOC, CLS], dt.float32, isOutput=True)

    slices = [nc.dram_tensor(f"slice_{l}", [SLICE_R, TW12 if l < 2 else TW3],
                             dt.bfloat16) for l in range(3)]
    tables = [[nc.dram_tensor(f"table_{l}_{j}", [RC, TW12 if l < 2 else TW3],
                              dt.bfloat16, addr_space="Shared")
               for j in range(J)] for l in range(3)]

    groups_rg = [list(range(NC))]

    with tile.TileContext(nc) as tc:
        with (
            tc.tile_pool(name="pers", bufs=1) as pers,
            tc.tile_pool(name="kt", bufs=3) as ktp,
            tc.tile_pool(name="stage", bufs=3) as stp,
            tc.tile_pool(name="gblk", bufs=2) as gp,
            tc.tile_pool(name="ohblk", bufs=2) as ohp,
            tc.tile_pool(name="small", bufs=3) as smp,
            tc.tile_pool(name="exR", bufs=2) as xp,
            tc.tile_pool(name="vals", bufs=5) as vp,
            tc.tile_pool(name="otile", bufs=2) as op_,
            tc.tile_pool(name="ps_feat", bufs=2, space="PSUM") as psf,
            tc.tile_pool(name="ps_out", bufs=2, space="PSUM") as pso,
            tc.tile_pool(name="ps_tr", bufs=2, space="PSUM") as pstr,
            tc.tile_pool(name="ps_er", bufs=2, space="PSUM") as pser,
        ):
            nc.gpsimd.load_library(mlp)
            # persistent SBUF state
            xT_sb = pers.tile([P, NBT * P], dt.bfloat16)
            nc.sync.dma_start(xT_sb[:], xT[:])
            w1_sb = pers.tile([P, D + 8], dt.bfloat16)
            nc.sync.dma_start(w1_sb[:], w1[:])
            w2_sb = pers.tile([P, 2 * (D + 8)], dt.bfloat16)
            w3_sb = pers.tile([P, 2 * (CLS + 2 + CLS)], dt.bfloat16)
            for kt in range(2):
                nc.sync.dma_start(w2_sb[:, kt * (D + 8):(kt + 1) * (D + 8)],
                                  w2[kt * P:(kt + 1) * P, :])
                nc.sync.dma_start(w3_sb[:, kt * (CLS + 2 + CLS):(kt + 1) * (CLS + 2 + CLS)],
                                  w3[kt * P:(kt + 1) * P, :])
            idx_sb = pers.tile([P, TT * 8], dt.int16)
            nc.sync.dma_start(idx_sb[:], idx_in[:])
            ident_sb = pers.tile([P, P], dt.bfloat16)
            nc.sync.dma_start(ident_sb[:], ident_in[:])
            h_sb = pers.tile([P, NBT * D], dt.bfloat16)
            er_sb = pers.tile([P, NBT * H12], dt.bfloat16)
            er3_sb = pers.tile([P, NBT], dt.bfloat16)
            res_sb = pers.tile([P, NBT * CLS], dt.float32)

            def _elu_into(x_t, dst_sb, b, width):
                # elu(x) = max(x, exp(min(x,0)) - 1)
                t1 = op_.tile([P, width], dt.float32, tag="elu1")
                nc.vector.tensor_scalar_min(t1[:], x_t[:], 0.0)
                nc.scalar.activation(t1[:], t1[:], mybir.ActivationFunctionType.Exp)
                nc.vector.tensor_scalar_add(t1[:], t1[:], -1.0)
                nc.vector.tensor_tensor(out=dst_sb[:, b * width:(b + 1) * width],
                                        in0=x_t[:], in1=t1[:], op=mybir.AluOpType.max)

            def feat_block(layer, nt):
                if layer == 0:
                    wsb, wcols, nk = w1_sb, D + 8, 1
                elif layer == 1:
                    wsb, wcols, nk = w2_sb, D + 8, 2
                else:
                    wsb, wcols, nk = w3_sb, CLS + 2 + CLS, 2
                f_ps = psf.tile([P, wcols], dt.float32, tag="fp")
                for kt in range(nk):
                    if layer == 0:
                        lhsT = xT_sb[:, nt * P:(nt + 1) * P]
                    else:
                        tr_ps = pstr.tile([P, P], dt.bfloat16, tag="trp")
                        nc.tensor.transpose(
                            tr_ps[:], h_sb[:, nt * D + kt * P: nt * D + (kt + 1) * P],
                            ident_sb[:])
                        ktile = ktp.tile([P, P], dt.bfloat16, tag="kt")
                        nc.scalar.activation(ktile[:], tr_ps[:],
                                             mybir.ActivationFunctionType.Copy)
                        lhsT = ktile[:]
                    nc.tensor.matmul(f_ps[:], lhsT, wsb[:, kt * wcols:(kt + 1) * wcols],
                                     start=(kt == 0), stop=(kt == nk - 1))
                rows = P if nt < NBT - 1 else LAST_ROWS
                if layer < 2:
                    st = stp.tile([P, D + H12], dt.bfloat16, tag="st")
                    nc.vector.tensor_copy(st[:], f_ps[:, 0:D + H12])
                    nc.vector.tensor_copy(er_sb[:, nt * H12:(nt + 1) * H12],
                                          f_ps[:, D + H12:D + 2 * H12])
                    nc.sync.dma_start(slices[layer][nt * P:nt * P + rows, 0:D + H12],
                                      st[0:rows, :])
                else:
                    st = stp.tile([P, CLS + 1], dt.bfloat16, tag="st3")
                    nc.vector.tensor_copy(st[:], f_ps[:, 0:CLS + 1])
                    nc.vector.tensor_copy(er3_sb[:, nt:nt + 1],
                                          f_ps[:, CLS + 1:CLS + 2])
                    nc.vector.tensor_copy(res_sb[:, nt * CLS:(nt + 1) * CLS],
                                          f_ps[:, CLS + 2:CLS + 2 + CLS])
                    nc.sync.dma_start(slices[2][nt * P:nt * P + rows, 0:CLS + 1],
                                      st[0:rows, :])

            def emit_ag(layer, j):
                nc.gpsimd.collective_compute(
                    "AllGather", mybir.AluOpType.bypass, replica_groups=groups_rg,
                    ins=[slices[layer][j * C:(j + 1) * C, :]],
                    outs=[tables[layer][j][:, :]])

            qn = [0]

            def edge_phase(layer):
                if layer < 2:
                    TW, FO, NH = TW12, D, H12
                    er_l = er_sb
                else:
                    TW, FO, NH = TW3, CLS, 1
                    er_l = er3_sb
                W2c = FO + NH          # vals row width
                tab = tables[layer]
                for gi, grp in enumerate(GROUPS):
                    gt0, gT = grange[gi]
                    ohE_g = ohp.tile([P, gT * P], dt.bfloat16, tag="ohE")
                    ohT_g = ohp.tile([P, gT * P], dt.bfloat16, tag="ohT")
                    nc.sync.dma_start(ohE_g[:], ohE_in[:, gt0 * P:(gt0 + gT) * P])
                    nc.sync.dma_start(ohT_g[:], ohT_in[:, gt0 * P:(gt0 + gT) * P])
                    outps = {}
                    for b in grp:
                        out_acc = pso.tile([P, W2c], dt.float32, tag="outp",
                                           name=f"outacc_{layer}_{gi}_{b}")
                        outps[b] = out_acc
                    vals_j = {}
                    for j in range(J):
                        j0, jT = gjr[(gi, j)]
                        if jT == 0:
                            continue
                        Gblk = gp.tile([P, jT * TW], dt.bfloat16, tag="G")
                        nc.gpsimd.dma_gather(
                            Gblk[:].rearrange("p (c e) -> p c e", c=jT),
                            tab[j][:, :],
                            idx_sb[:, j0 * 8:(j0 + jT) * 8],
                            jT * P, jT * P, TW, queue_num=qn[0] % 4,
                        )
                        qn[0] += 1
                        er_ps = pser.tile([P, jT * NH], dt.float32, tag="erp")
                        for b in grp:
                            s0, n = tid[(b, j)]
                            for t in range(s0, s0 + n):
                                lo = t - gt0
                                nc.tensor.matmul(
                                    er_ps[:, (t - j0) * NH:(t - j0 + 1) * NH],
                                    ohT_g[:, lo * P:(lo + 1) * P],
                                    er_l[:, b * NH:(b + 1) * NH],
                                    start=True, stop=True)
                        e_sb = smp.tile([P, jT * NH], dt.float32, tag="e")
                        nc.vector.tensor_tensor(
                            out=e_sb[:],
                            in0=Gblk[:].rearrange("p (t c) -> p t c", t=jT)[:, :, FO:FO + NH],
                            in1=er_ps[:, :jT * NH],
                            op=mybir.AluOpType.add,
                        )
                        es_sb = smp.tile([P, jT * NH], dt.float32, tag="es")
                        nc.vector.tensor_scalar_mul(es_sb[:], e_sb[:], SLOPE)
                        nc.vector.tensor_tensor(out=e_sb[:], in0=e_sb[:], in1=es_sb[:],
                                                op=mybir.AluOpType.max)
                        ex_sb = smp.tile([P, jT * NH], dt.float32, tag="ex")
                        nc.scalar.activation(ex_sb[:], e_sb[:],
                                             mybir.ActivationFunctionType.Exp)
                        exR = xp.tile([P, jT * W2c], dt.bfloat16, tag="xr")
                        nc.scalar.activation(
                            exR[:].rearrange("p (t c) -> p t c", t=jT)[:, :, 0:FO]
                                .rearrange("p t (h f) -> p t h f", h=NH),
                            ex_sb[:].rearrange("p (t h o) -> p t h o", t=jT, o=1)
                                .to_broadcast([P, jT, NH, F]),
                            mybir.ActivationFunctionType.Copy,
                        )
                        nc.scalar.activation(
                            exR[:].rearrange("p (t c) -> p t c", t=jT)[:, :, FO:FO + NH],
                            ex_sb[:].rearrange("p (t h) -> p t h", t=jT),
                            mybir.ActivationFunctionType.Copy,
                        )
                        nc.vector.memset(
                            Gblk[:].rearrange("p (t c) -> p t c", t=jT)[:, :, FO:FO + NH],
                            1.0)
                        vals = vp.tile([P, jT * W2c], dt.bfloat16, tag="v")
                        nc.vector.tensor_tensor(
                            out=vals[:].rearrange("p (t c) -> p t c", t=jT),
                            in0=Gblk[:].rearrange("p (t c) -> p t c", t=jT)[:, :, 0:W2c],
                            in1=exR[:].rearrange("p (t c) -> p t c", t=jT),
                            op=mybir.AluOpType.mult,
                        )
                        vals_j[j] = vals
                    # contiguous PSUM accumulation group per block
                    for b in grp:
                        for j in range(J):
                            j0, jT = gjr[(gi, j)]
                            s0, n = tid[(b, j)]
                            for t in range(s0, s0 + n):
                                lo = t - gt0
                                nc.tensor.matmul(
                                    outps[b][:],
                                    ohE_g[:, lo * P:(lo + 1) * P],
                                    vals_j[j][:, (t - j0) * W2c:(t - j0 + 1) * W2c],
                                    start=(t == bfirst[b]), stop=(t == blast[b]))
                    for b in grp:
                        out_ps = outps[b]
                        den = smp.tile([P, NH], dt.float32, tag="den")
                        nc.vector.tensor_scalar_max(den[:], out_ps[:, FO:FO + NH], 1e-30)
                        rec = smp.tile([P, NH], dt.float32, tag="rec")
                        nc.vector.reciprocal(rec[:], den[:])
                        o_t = op_.tile([P, FO], dt.float32, tag="ot")
                        nc.vector.tensor_tensor(
                            out=o_t[:].rearrange("p (h f) -> p h f", h=NH),
                            in0=out_ps[:, 0:FO].rearrange("p (h f) -> p h f", h=NH),
                            in1=rec[:].to_broadcast([P, NH, F]),
                            op=mybir.AluOpType.mult,
                        )
                        # layer tails
                        if layer == 0:
                            _elu_into(o_t, h_sb, b, D)
                        elif layer == 1:
                            pre = op_.tile([P, D], dt.float32, tag="pre")
                            nc.vector.tensor_tensor(out=pre[:], in0=o_t[:],
                                                    in1=h_sb[:, b * D:(b + 1) * D],
                                                    op=mybir.AluOpType.add)
                            _elu_into(pre, h_sb, b, D)
                        else:
                            lg = op_.tile([P, CLS], dt.float32, tag="lg")
                            nc.vector.tensor_tensor(out=lg[:], in0=o_t[:],
                                                    in1=res_sb[:, b * CLS:(b + 1) * CLS],
                                                    op=mybir.AluOpType.add)
                            rows = P if b < NBT - 1 else LAST_ROWS
                            nc.sync.dma_start(outp[b * P:b * P + rows, :], lg[0:rows, :])
                        if layer < 2:
                            feat_block(layer + 1, b)

            for b in range(NBT):
                feat_block(0, b)
            for j in range(J):
                emit_ag(0, j)
            for layer in range(n_layers):
                edge_phase(layer)
                if layer < 2:
                    for j in range(J):
                        emit_ag(layer + 1, j)

    nc.compile()
    return nc


LAST_RESULTS = None


def prepare(inputs):
    import ml_dtypes
    bf16 = ml_dtypes.bfloat16

    x = np.asarray(inputs["x"], np.float32)
    src = np.asarray(inputs["src"]).astype(np.int64)
    dst = np.asarray(inputs["dst"]).astype(np.int64)
    W1 = np.asarray(inputs["W1"], np.float32)
    W2 = np.asarray(inputs["W2"], np.float32)
    W3 = np.asarray(inputs["W3"], np.float32)
    res_W3 = np.asarray(inputs["res_W3"], np.float32)
    al1 = np.asarray(inputs["al1"], np.float32)
    ar1 = np.asarray(inputs["ar1"], np.float32)
    al2 = np.asarray(inputs["al2"], np.float32)
    ar2 = np.asarray(inputs["ar2"], np.float32)
    al3 = np.asarray(inputs["al3"], np.float32)
    ar3 = np.asarray(inputs["ar3"], np.float32)

    def ext(W, al, ar, nh, res=None):
        Wr = W.reshape(W.shape[0], nh, -1)
        wel = np.einsum("khf,hf->kh", Wr, al)
        wer = np.einsum("khf,hf->kh", Wr, ar)
        parts = [W, wel, wer] + ([res] if res is not None else [])
        return np.ascontiguousarray(np.concatenate(parts, axis=1), dtype=bf16)

    w1e = ext(W1, al1, ar1, H12)                 # [128, 264]
    w2e = ext(W2, al2, ar2, H12)                 # [256, 264]
    w3e = ext(W3, al3, ar3, 1, res_W3)           # [256, 130]

    tile_block, tile_stripe, TT, idx16, dstpos = make_schedule(src, dst)
    nc = build_nc(tile_block, tile_stripe, TT)

    ident = np.eye(P, dtype=bf16)

    in_maps = []
    for k in range(NC):
        xk = x[k * NLOC:(k + 1) * NLOC].T                     # [128, 6250]
        xk = np.pad(xk, ((0, 0), (0, NBT * P - NLOC)))
        dp = dstpos[k]                                        # [128, TT]
        ohE = (dp[:, :, None] == np.arange(P, dtype=np.float32)[None, None, :])
        ohE = ohE.astype(bf16)                                # [e, t, d]
        ohT = np.ascontiguousarray(ohE.transpose(2, 1, 0))    # [d, t, e]
        in_maps.append({
            "xT": np.ascontiguousarray(xk).astype(bf16),
            "w1": w1e, "w2": w2e, "w3": w3e,
            "idx16": np.ascontiguousarray(idx16[k]),
            "ohE": np.ascontiguousarray(ohE.reshape(P, len(tile_block) * P)),
            "ohT": ohT.reshape(P, len(tile_block) * P),
            "ident": ident,
        })
    return nc, in_maps


def kernel(**inputs):
    from concourse.bass_utils import run_bass_kernel_spmd

    nc, in_maps = prepare(inputs)
    res = run_bass_kernel_spmd(nc, in_maps, core_ids=list(range(NC)))
    global LAST_RESULTS
    LAST_RESULTS = res
    out = np.concatenate([res.results[k]["out"] for k in range(NC)], axis=0)
    return out.astype(np.float32)
